# revision 1
# baseline (speedup 1.0000x reference)
"""Trainium2 Bass kernel for ColumnParallelLinearWithTopping.

Computes  y[t] = x[t] @ (W_base.T + DeltaW[j] + A[j] @ B[j]),  j = weight_indices[t]

Strategy (8-core tensor parallel over the output dim, 512 cols/core):
  * Host: stable-argsort tokens by adapter id, pack x rows grouped by
    adapter (each group padded to a multiple of 128 rows), and ship x
    TRANSPOSED ([D_IN, T_pad]) so the device never transposes activations.
    W_base is pre-transposed to [D_IN, D_OUT]; A to [RANK, D_IN]
    (layout-only transforms).
  * Device (per core, SPMD): for each adapter a, build the effective
    weight  W_full[a] = W_base.T + DeltaW[a] + A[a] @ B[a]  (column shard)
    in SBUF (PE matmul for A@B + DVE adds), then a grouped GEMM over that
    adapter's tokens, 6 blocks (768 tokens) at a time:
        psum_y[b][tok,512] += xT[k, tokens_b].T @ W_full[a][k]
    All matmuls run in float32r (full-rate fp32 PE mode).
  * Host: concatenate per-core column shards and undo the permutation.
"""
import os
from contextlib import ExitStack

import numpy as np

import concourse.bass as bass
import concourse.mybir as mybir
import concourse.tile as tile
from concourse import bacc
from concourse.bass_utils import run_bass_kernel_spmd

T, D_IN, D_OUT = 8192, 4096, 4096
N_ADAPT, RANK = 8, 16
N_CORES = 8
P = 128
SHARD = D_OUT // N_CORES          # 512 output cols per core
KT = D_IN // P                    # 32 contraction tiles
GRP = 6                           # token blocks per GEMM group
F32 = mybir.dt.float32
F32R = mybir.dt.float32r

USE_F32R = os.environ.get("KERNEL_FP32R", "1") == "1"
DT = F32R if USE_F32R else F32   # dtype of the x / weight data path

_build_cache: dict = {}


def _build(nb: tuple, nvalid: tuple):
    """Build + compile the SPMD program for per-adapter block counts nb."""
    t_pad = sum(nb) * P
    nc = bacc.Bacc("TRN2", target_bir_lowering=False, debug=False)
    xt = nc.dram_tensor("xt", [D_IN, t_pad], DT, kind="ExternalInput").ap()
    wbt = nc.dram_tensor("wbt", [P, KT * SHARD], DT, kind="ExternalInput").ap()
    dw = nc.dram_tensor("dw", [N_ADAPT, KT // 4, P, 4 * SHARD], DT, kind="ExternalInput").ap()
    at = nc.dram_tensor("at", [N_ADAPT, RANK, D_IN], DT, kind="ExternalInput").ap()
    bb = nc.dram_tensor("bb", [N_ADAPT, RANK, SHARD], DT, kind="ExternalInput").ap()
    y = nc.dram_tensor("y", [t_pad, SHARD], F32, kind="ExternalOutput").ap()

    with tile.TileContext(nc) as tc, ExitStack() as ctx:
        const = ctx.enter_context(tc.tile_pool(name="const", bufs=1))
        wf_pool = ctx.enter_context(tc.tile_pool(name="wf", bufs=34))
        xt_pool = ctx.enter_context(tc.tile_pool(name="xtp", bufs=6))
        dwt_pool = ctx.enter_context(tc.tile_pool(name="dwt", bufs=2))
        ab_pool = ctx.enter_context(tc.tile_pool(name="ab", bufs=1))
        y_pool = ctx.enter_context(tc.tile_pool(name="yo", bufs=3))
        psum_y = ctx.enter_context(tc.tile_pool(name="psum_y", bufs=1, space="PSUM"))
        psum_m = ctx.enter_context(tc.tile_pool(name="psum_m", bufs=2, space="PSUM"))

        wbt_sb = const.tile([P, KT, SHARD], DT, name="wbt_sb")
        nc.scalar.dma_start(wbt_sb, wbt.rearrange("p (kt n) -> p kt n", kt=KT))

        blk_base = 0
        for a in range(N_ADAPT):
            if nb[a] == 0:
                continue
            at_sb = ab_pool.tile([RANK, D_IN], DT, name="at_sb")
            nc.scalar.dma_start(at_sb, at[a])
            b_sb = ab_pool.tile([RANK, SHARD], DT, name="b_sb")
            nc.scalar.dma_start(b_sb, bb[a])

            # ---- build W_full[a] in SBUF: 32 tiles of [128, SHARD] ----
            wtiles = []
            for k4 in range(KT // 4):
                dwt = dwt_pool.tile([P, 4, SHARD], DT, name="dwt")
                nc.scalar.dma_start(
                    dwt, dw[a, k4].rearrange("p (i n) -> p i n", i=4))
                for i in range(4):
                    k = k4 * 4 + i
                    ab_ps = psum_m.tile([P, SHARD], F32, name="ab_ps")
                    nc.tensor.matmul(
                        ab_ps,
                        at_sb[:, k * P:(k + 1) * P],
                        b_sb,
                        start=True, stop=True,
                    )
                    wf = wf_pool.tile([P, SHARD], DT, name="wf")
                    nc.vector.tensor_add(wf, ab_ps, dwt[:, i, :])
                    nc.vector.tensor_add(wf, wf, wbt_sb[:, k, :])
                    wtiles.append(wf)

            # ---- grouped GEMM: up to GRP token blocks at a time ----
            blk = 0
            while blk < nb[a]:
                g = min(GRP, nb[a] - blk)
                tok0 = (blk_base + blk) * P
                W = g * P
                psums = [psum_y.tile([P, SHARD], F32, name=f"py{b}",
                                     tag=f"py{b}", bufs=1)
                         for b in range(g)]
                for k in range(KT):
                    xt_sb = xt_pool.tile([P, GRP * P], DT, name="xt_sb")
                    nc.sync.dma_start(
                        xt_sb[:, :W], xt[k * P:(k + 1) * P, tok0:tok0 + W])
                    for b in range(g):
                        nc.tensor.matmul(
                            psums[b],
                            xt_sb[:, b * P:(b + 1) * P],
                            wtiles[k],
                            start=(k == 0), stop=(k == KT - 1),
                        )
                for b in range(g):
                    y_sb = y_pool.tile([P, SHARD], F32, name="y_sb")
                    nc.vector.tensor_copy(y_sb, psums[b])
                    nc.scalar.dma_start(
                        y[tok0 + b * P:tok0 + (b + 1) * P, :], y_sb)
                blk += g
            blk_base += nb[a]

    nc.compile()
    return nc, t_pad


def kernel(x, weight_indices, W_base, A_buffer, B_buffer, DeltaW):
    x = np.asarray(x, dtype=np.float32)
    idx = np.asarray(weight_indices).astype(np.int64)
    W_base = np.asarray(W_base, dtype=np.float32)
    A_buffer = np.asarray(A_buffer, dtype=np.float32)
    B_buffer = np.asarray(B_buffer, dtype=np.float32)
    DeltaW = np.asarray(DeltaW, dtype=np.float32)

    order = np.argsort(idx, kind="stable")
    counts = np.bincount(idx, minlength=N_ADAPT)
    nb = tuple(int(-(-c // P)) for c in counts)
    t_pad = sum(nb) * P

    nvalid = tuple(int(c) for c in counts)
    key = (nb, nvalid)
    if key not in _build_cache:
        _build_cache[key] = _build(nb, nvalid)
    nc, _ = _build_cache[key]

    # pack x columns (transposed) grouped by adapter, pad to 128-row blocks
    xT = np.ascontiguousarray(x.T)                  # [D_IN, T]
    xt_packed = np.zeros((D_IN, t_pad), dtype=np.float32)
    seg_dst = []          # (dst_row0, count, sorted_token_slice_start)
    cum = np.concatenate([[0], np.cumsum(counts)])
    row0 = 0
    for a in range(N_ADAPT):
        c = int(counts[a])
        if c:
            xt_packed[:, row0:row0 + c] = xT[:, order[cum[a]:cum[a] + c]]
        seg_dst.append((row0, c, int(cum[a])))
        row0 += nb[a] * P

    wbT = np.ascontiguousarray(W_base.T)                     # [D_IN, D_OUT]
    # [D_OUT/SHARD][P, KT*SHARD]: partition-major so DMA rows are contiguous
    wb_r = wbT.reshape(KT, P, D_OUT).transpose(1, 0, 2)      # [P, KT, D_OUT]
    atT = np.ascontiguousarray(A_buffer.transpose(0, 2, 1))  # [A, RANK, D_IN]

    in_maps = []
    for c in range(N_CORES):
        sl = slice(c * SHARD, (c + 1) * SHARD)
        in_maps.append({
            "xt": xt_packed,
            "wbt": np.ascontiguousarray(
                wb_r[:, :, sl]).reshape(P, KT * SHARD),
            "dw": np.ascontiguousarray(
                DeltaW[:, :, sl].reshape(N_ADAPT, KT // 4, 4, P, SHARD)
                .transpose(0, 1, 3, 2, 4)).reshape(
                    N_ADAPT, KT // 4, P, 4 * SHARD),
            "at": atT,
            "bb": np.ascontiguousarray(B_buffer[:, :, sl]),
        })

    global _last_in_maps
    _last_in_maps = in_maps
    res = run_bass_kernel_spmd(nc, in_maps, core_ids=list(range(N_CORES)))
    y_packed = np.concatenate(
        [res.results[c]["y"] for c in range(N_CORES)], axis=1)  # [t_pad, D_OUT]

    out = np.empty((T, D_OUT), dtype=np.float32)
    for a in range(N_ADAPT):
        row0, c, s = seg_dst[a]
        if c:
            out[order[s:s + c]] = y_packed[row0:row0 + c]
    return out



# revision 2
# speedup vs baseline: 1.0008x; 1.0008x over previous
"""Trainium2 Bass kernel for ColumnParallelLinearWithTopping.

Computes  y[t] = x[t] @ (W_base.T + DeltaW[j] + A[j] @ B[j]),  j = weight_indices[t]

Strategy (8-core adapter-parallel, host-premerged weights, bf16 data path):
  * Host: W_eff[a] = W_base.T + DeltaW[a] + A[a] @ B[a]  (f32 -> bf16),
    tokens stable-sorted by adapter.  Core a gets adapter a's tokens
    (padded to T_PAD) and the full [4096, 4096] W_eff[a].
  * Device (per core): x^T resident in SBUF ([128, KT, T_PAD] bf16, ~9 MB).
    Stream W_eff once (32 col-blocks x 1 MB), mapping:
        psum[col128, tok] += W_tile[k, col128].T @ x^T[k, tok]
    fp32 PSUM accumulation over the 32 k-tiles; moving chunks of 512 tokens
    (one PSUM bank each, double-buffered across col-blocks).
    y^T written back as bf16 [32, 128, T_PAD].
  * Host: transpose y^T shards back and undo the token permutation.
"""
from contextlib import ExitStack

import numpy as np
import ml_dtypes

import concourse.bass as bass
import concourse.mybir as mybir
import concourse.tile as tile
from concourse import bacc
from concourse.bass_utils import run_bass_kernel_spmd

T, D_IN, D_OUT = 8192, 4096, 4096
N_ADAPT, RANK = 8, 16
N_CORES = 8
P = 128
KT = D_IN // P                    # 32 contraction tiles
CB = D_OUT // P                   # 32 column blocks
F32 = mybir.dt.float32
BF16 = mybir.dt.bfloat16
NPBF16 = ml_dtypes.bfloat16

_build_cache: dict = {}
_last_in_maps = None


def _chunk_widths(t_pad: int):
    """Split t_pad into <=512-wide chunks (PSUM bank limit), as evenly as
    possible in multiples of 16 so no matmul stream is too short to hide
    its LDWEIGHTS."""
    n = -(-t_pad // 512)
    widths = []
    rem = t_pad
    for i in range(n, 0, -1):
        w = min(512, ((rem + i - 1) // i + 15) & ~15, rem)
        widths.append(w)
        rem -= w
    assert sum(widths) == t_pad and all(w <= 512 for w in widths)
    return widths


def _build(t_pad: int):
    """Build + compile the SPMD program for per-core token count t_pad."""
    widths = _chunk_widths(t_pad)
    chunks = []
    t0 = 0
    for w in widths:
        chunks.append((t0, w))
        t0 += w
    assert len(chunks) <= 4, f"t_pad={t_pad} needs >4 PSUM chunks"

    nc = bacc.Bacc("TRN2", target_bir_lowering=False, debug=False)
    xk = nc.dram_tensor("xk", [P, KT, t_pad], BF16, kind="ExternalInput").ap()
    wk = nc.dram_tensor("wk", [CB, P, KT, P], BF16, kind="ExternalInput").ap()
    y = nc.dram_tensor("y", [CB, P, t_pad], BF16, kind="ExternalOutput").ap()

    with tile.TileContext(nc) as tc, ExitStack() as ctx:
        xpool = ctx.enter_context(tc.tile_pool(name="xp", bufs=1))
        wpool = ctx.enter_context(tc.tile_pool(name="wp", bufs=3))
        ypool = ctx.enter_context(tc.tile_pool(name="yp", bufs=3))
        pspool = ctx.enter_context(tc.tile_pool(name="ps", bufs=2, space="PSUM"))

        # resident x^T: [128, 32, t_pad] bf16; tiny leading chunks so the
        # first matmuls start as early as possible
        xt = xpool.tile([P, KT, t_pad], BF16, name="xt")
        x_chunks = [(0, 1), (1, 1), (2, 2)] + [(kc, 2) for kc in range(4, KT, 2)]
        for kc, kw in x_chunks:
            nc.sync.dma_start(xt[:, kc:kc + kw, :], xk[:, kc:kc + kw, :])

        # col-blocks 0+1 interleaved at k granularity: during the x preload
        # window (DMA-bandwidth-bound) the PE has 2x the work per x k-chunk,
        # so it doesn't outrun the x DMA stream and stall.
        wts, pss = [], []
        for cb in range(2):
            wt = wpool.tile([P, KT, P], BF16, name="wt")
            for kc in range(0, KT, 4):   # k-ordered pieces: k=0 weights land fast
                nc.scalar.dma_start(wt[:, kc:kc + 4, :],
                                    wk[cb, :, kc:kc + 4, :])
            wts.append(wt)
            pss.append([pspool.tile([P, 512], F32, name=f"ps{i}", tag=f"ps{i}")
                        for i in range(len(chunks))])
        for k in range(KT):
            for cb in range(2):
                for i, (t0, w) in enumerate(chunks):
                    nc.tensor.matmul(
                        pss[cb][i][:, :w], wts[cb][:, k, :], xt[:, k, t0:t0 + w],
                        start=(k == 0), stop=(k == KT - 1),
                    )
        for cb in range(2):
            yt = ypool.tile([P, t_pad], BF16, name="yt")
            for i, (t0, w) in enumerate(chunks):
                nc.vector.tensor_copy(yt[:, t0:t0 + w], pss[cb][i][:, :w])
            nc.sync.dma_start(y[cb], yt)

        for cb in range(2, CB):
            wt = wpool.tile([P, KT, P], BF16, name="wt")
            nc.scalar.dma_start(wt, wk[cb])

            psums = [pspool.tile([P, 512], F32, name=f"ps{i}", tag=f"ps{i}")
                     for i in range(len(chunks))]
            yt = ypool.tile([P, t_pad], BF16, name="yt")
            if cb < CB - 1:
                for k in range(KT):
                    lhsT = wt[:, k, :]
                    for i, (t0, w) in enumerate(chunks):
                        nc.tensor.matmul(
                            psums[i][:, :w], lhsT, xt[:, k, t0:t0 + w],
                            start=(k == 0), stop=(k == KT - 1),
                        )
                for i, (t0, w) in enumerate(chunks):
                    nc.vector.tensor_copy(yt[:, t0:t0 + w], psums[i][:, :w])
                nc.sync.dma_start(y[cb], yt)
            else:
                # last col-block: chunk-outer so each chunk's cast + store
                # overlaps the remaining chunks' matmuls (shorter tail)
                for i, (t0, w) in enumerate(chunks):
                    for k in range(KT):
                        nc.tensor.matmul(
                            psums[i][:, :w], wt[:, k, :], xt[:, k, t0:t0 + w],
                            start=(k == 0), stop=(k == KT - 1),
                        )
                    nc.vector.tensor_copy(yt[:, t0:t0 + w], psums[i][:, :w])
                    nc.sync.dma_start(y[cb, :, t0:t0 + w], yt[:, t0:t0 + w])

    nc.compile()
    return nc, t_pad


def kernel(x, weight_indices, W_base, A_buffer, B_buffer, DeltaW):
    global _last_in_maps
    x = np.asarray(x, dtype=np.float32)
    idx = np.asarray(weight_indices).astype(np.int64)
    W_base = np.asarray(W_base, dtype=np.float32)
    A_buffer = np.asarray(A_buffer, dtype=np.float32)
    B_buffer = np.asarray(B_buffer, dtype=np.float32)
    DeltaW = np.asarray(DeltaW, dtype=np.float32)

    order = np.argsort(idx, kind="stable")
    counts = np.bincount(idx, minlength=N_ADAPT)
    cum = np.concatenate([[0], np.cumsum(counts)])
    t_pad = max(64, int(-(-counts.max() // 16)) * 16)

    if t_pad not in _build_cache:
        _build_cache[t_pad] = _build(t_pad)
    nc, _ = _build_cache[t_pad]

    WbT = np.ascontiguousarray(W_base.T)            # [D_IN, D_OUT]
    in_maps = []
    tok_lists = []
    for a in range(N_ADAPT):
        toks = order[cum[a]:cum[a + 1]]
        tok_lists.append(toks)
        Weff = WbT + DeltaW[a] + A_buffer[a] @ B_buffer[a]
        # [cb, p, k, c]: per-colblock contiguous [128, 32*128] DMA chunks
        wkb = np.ascontiguousarray(
            Weff.astype(NPBF16).reshape(KT, P, CB, P).transpose(2, 1, 0, 3))
        xtp = np.zeros((D_IN, t_pad), dtype=np.float32)
        xtp[:, :len(toks)] = x[toks].T
        xkb = np.ascontiguousarray(
            xtp.astype(NPBF16).reshape(KT, P, t_pad).transpose(1, 0, 2))
        in_maps.append({"xk": xkb, "wk": wkb})

    _last_in_maps = in_maps
    res = run_bass_kernel_spmd(nc, in_maps, core_ids=list(range(N_CORES)))

    out = np.empty((T, D_OUT), dtype=np.float32)
    for a in range(N_ADAPT):
        c = len(tok_lists[a])
        yk = np.asarray(res.results[a]["y"]).reshape(D_OUT, t_pad)
        out[tok_lists[a]] = yk[:, :c].T.astype(np.float32)
    return out
